# revision 9
# baseline (speedup 1.0000x reference)
"""Trainium2 Bass kernel for the Noisy-Weights BNN MLP.

Computation (full problem):
  noise1[0] = 0;  W1n = W1[None] + noise1            # [16, 512, 512]
  X = sigmoid(A @ W0)        A = batch.reshape(2048, 784)
  Y_s = sigmoid(X @ W1n[s])
  Z_s = sigmoid(Y_s @ W2)    -> out [16, 32, 64, 10]

Sharding over 8 NeuronCores: 2 replica-groups (8 replicas each) x
4 token-groups (512 tokens each).  Each core redundantly computes the
shared layer 0 for its 512 tokens, then its 8 replicas of layers 1+2.

All three layers run in fp8e4m3 with DoubleRow perf mode (2 k-tiles per
pass).  Accuracy is preserved by storing the hidden activation Y in
*centered* form: the layer-1 activation computes y2 = tanh(0.5*ps) =
2*sigmoid(ps)-1, which quantizes to fp8 with half the absolute error of
sigmoid outputs clustered near 1.  Layer 2 then computes y2 @ W2 and the
host finishes with sigmoid(0.5*zt + 0.5*colsum(W2q)).  Simulated
end-to-end rel-L2 error vs the fp32 reference: ~8e-3.

On-device layout: every matmul is out = lhsT.T @ rhs with contraction on
SBUF partitions:
  layer0: lhsT = W0 [128, 2, 128m], rhs = A^T [128, 2, 512] (k-pairs,
          784 zero-padded to 1024 = 4 pairs) -> psum X^T, sigmoid->fp8
  layer1: lhsT = W1n pair,          rhs = X^T pair -> psum, tanh->fp8
  layer2: lhsT = W2 pair [128,2,10], rhs = Y^T pair, both pairs
          accumulate in one PSUM bank -> single DVE copy to bf16

Schedule notes: short dummy matmuls warm the PE clock (HAM) while the
first DMA chunk lands; layer-0 A^T/W0 are packed interleaved per k-pair
and DMA'd in 4 chunks; each replica's layer-2 pair is issued one m-pair
*after* its activation so the PE FIFO never stalls.
"""

import os
import sys

import numpy as np
import ml_dtypes

if "/opt/trn_rl_repo" not in sys.path:
    sys.path.insert(0, "/opt/trn_rl_repo")

import concourse.bass as bass  # noqa: E402
import concourse.tile as tile  # noqa: E402
from concourse import bacc, mybir  # noqa: E402
from concourse.bass_utils import run_bass_kernel_spmd  # noqa: E402

# ---- problem constants (hardcoded; kernel.py must be self-contained) ----
S = 16           # noisy-weight replicas
BT = 2048        # batch tokens = 32 * 64
D_IN = 784
D_H = 512
D_OUT = 10
KA = 1024        # 784 zero-padded to 8 * 128 (4 DoubleRow k-pairs)
N_CORES = 8
SG = 2           # replica groups
TG = 4           # token groups
R_LOC = S // SG          # replicas per core = 8
NT = BT // TG            # tokens per core = 512
KK0 = KA // 256          # 4 k-pairs for layer 0
KH_T = D_H // 128        # 4 k-tiles for hidden dims (2 pairs)

BF16 = mybir.dt.bfloat16
FP8 = mybir.dt.float8e4
F32 = mybir.dt.float32
DR = mybir.MatmulPerfMode.DoubleRow

# layer-2 matmul mode: "dr16" = DoubleRow pairs with W2 zero-padded to 16
# output cols, "plain" = 4 plain fp8 matmuls (ISA forbids dual-fp8
# ldweights at 10 cols)
L2_MODE = os.environ.get("KERNEL_L2_MODE", "dr16")
M2 = 16 if L2_MODE == "dr16" else D_OUT   # layer-2 packed output cols

_CACHE = {}

last_results = None  # BassKernelResults of the most recent run (for test.py)


def _build_program():
    """One SPMD Bass program; per-core differences live entirely in data."""
    nc = bacc.Bacc(None, target_bir_lowering=False, debug=False,
                   enable_partition_id=False)

    # layer-0 inputs interleaved per k-pair:
    # aw[:, kk*2048+0:1024]    = A^T pair [2, 512] (fp8)
    # aw[:, kk*2048+1024:2048] = W0  pair [2, 512] (fp8)
    aw_d = nc.dram_tensor("aw_pack", [128, KK0 * 2048], FP8,
                          kind="ExternalInput")
    w1_d = nc.dram_tensor("w1_pack", [128, R_LOC * KH_T * D_H], FP8,
                          kind="ExternalInput")
    w2_d = nc.dram_tensor("w2_pack", [128, KH_T * M2], FP8,
                          kind="ExternalInput")
    zt_d = nc.dram_tensor("zt", [D_OUT, R_LOC * NT], BF16,
                          kind="ExternalOutput")

    SIG = mybir.ActivationFunctionType.Sigmoid
    TANH = mybir.ActivationFunctionType.Tanh

    with tile.TileContext(nc) as tc:
        with (
            tc.tile_pool(name="consts", bufs=1) as consts,
            tc.tile_pool(name="w1p", bufs=1) as w1p,
            tc.tile_pool(name="yp", bufs=3) as yp,
            tc.tile_pool(name="px", bufs=3, space="PSUM") as px,
            tc.tile_pool(name="pz", bufs=2, space="PSUM") as pz,
        ):
            warm_sb = consts.tile([128, 256], FP8)
            aw_sb = consts.tile([128, KK0 * 2048], FP8)
            w2_sb = consts.tile([128, KH_T * M2], FP8)
            x_sb = consts.tile([128, KH_T * NT], FP8)
            z_sb = consts.tile([D_OUT, R_LOC * NT], BF16)

            def at_kk(kk):
                return aw_sb[:, kk * 2048:kk * 2048 + 1024].rearrange(
                    "p (a n) -> p a n", a=2)

            def w0_kk(kk):
                return aw_sb[:, kk * 2048 + 1024:(kk + 1) * 2048].rearrange(
                    "p (a n) -> p a n", a=2)

            def w2_kp(kp):
                return w2_sb[:, kp * 2 * M2:(kp + 1) * 2 * M2].rearrange(
                    "p (a m) -> p a m", a=2)

            # Input DMA triggers first, spread across the three DGE-capable
            # queues (SP, Activation, Pool) so descriptor generation (~0.6 us
            # apiece) runs in parallel and the transfers start early.
            def aw_dma(eng, kk):
                eng.dma_start(out=aw_sb[:, kk * 2048:(kk + 1) * 2048],
                              in_=aw_d[:, kk * 2048:(kk + 1) * 2048])

            w1_sb = [w1p.tile([128, 2 * KH_T * D_H], FP8, name=f"w1c{ci}")
                     for ci in range(4)]

            def w1_dma(eng, ci):
                eng.dma_start(
                    out=w1_sb[ci][:],
                    in_=w1_d[:, ci * 2 * KH_T * D_H:(ci + 1) * 2 * KH_T * D_H])

            nc.gpsimd.memset(warm_sb[:], 0)   # first: warmups need it
            aw_dma(nc.sync, 0)
            aw_dma(nc.scalar, 1)
            aw_dma(nc.sync, 2)
            aw_dma(nc.scalar, 3)
            w1_dma(nc.sync, 0)
            nc.scalar.dma_start(out=w2_sb[:], in_=w2_d[:])
            w1_dma(nc.gpsimd, 1)
            w1_dma(nc.sync, 2)
            w1_dma(nc.gpsimd, 3)

            # PE warm-up: short dummy matmuls keep TensorE busy (and
            # un-throttle the HAM clock gate) while the first input DMA
            # lands; short so layer 0 isn't stuck behind them in the FIFO.
            wps = px.tile([128, 1024], F32, name="ps")
            for _ in range(9):
                nc.tensor.matmul(wps[:, :256], lhsT=warm_sb[:, :128],
                                 rhs=warm_sb[:], start=True, stop=True)

            # ---- layer 0: X^T = sigmoid(W0^T A^T), DoubleRow k-pairs ----
            # mp0 strictly first so its sigmoid (which feeds replica 0's
            # k0-pair matmuls) fires as early as possible; kk-outer so early
            # k-pair chunks are consumed while later chunks are in flight.
            for mp in range(2):           # m pairs: (0,1), (2,3)
                ps = px.tile([128, 1024], F32, name="ps")
                for kk in range(KK0):
                    for m2 in range(2):
                        m = 2 * mp + m2
                        nc.tensor.matmul(
                            ps[:, m2 * NT:(m2 + 1) * NT],
                            lhsT=w0_kk(kk)[:, :, m * 128:(m + 1) * 128],
                            rhs=at_kk(kk),
                            start=(kk == 0),
                            stop=(kk == KK0 - 1),
                            perf_mode=DR,
                        )
                nc.scalar.activation(
                    x_sb[:, mp * 1024:(mp + 1) * 1024], ps[:], SIG)

            # ---- per replica: layer 1, with layer 2 deferred-interleaved ----
            psz = {}

            def l2(r):
                # all k-tiles accumulate into one PSUM bank
                if L2_MODE == "dr16":
                    for kp in range(2):
                        nc.tensor.matmul(
                            psz[r][0:M2, :],
                            lhsT=w2_kp(kp),
                            rhs=y3s[r][:, 2 * kp:2 * kp + 2, :],
                            start=(kp == 0), stop=(kp == 1),
                            perf_mode=DR,
                        )
                else:
                    for k in range(KH_T):
                        nc.tensor.matmul(
                            psz[r][0:D_OUT, :],
                            lhsT=w2_sb[:, k * M2:k * M2 + D_OUT],
                            rhs=y_sbs[r][:, k * NT:(k + 1) * NT],
                            start=(k == 0), stop=(k == KH_T - 1),
                        )

            def l2_reduce(r):
                nc.vector.tensor_copy(out=z_sb[:, r * NT:(r + 1) * NT],
                                      in_=psz[r][0:D_OUT, :])
                psz.pop(r)

            y_sbs = {}
            y3s = {}
            x3 = x_sb[:].rearrange("p (k n) -> p k n", k=KH_T)
            for r in range(R_LOC):
                w1c = w1_sb[r // 2]
                roff = (r % 2) * KH_T * D_H
                w1c3 = w1c[:, roff:roff + KH_T * D_H].rearrange(
                    "p (k n) -> p k n", k=KH_T)
                y_sbs[r] = yp.tile([128, KH_T * NT], FP8, name="y_sb")
                y3s[r] = y_sbs[r][:].rearrange("p (k n) -> p k n", k=KH_T)
                psz[r] = pz.tile([128, NT], F32, name="psz")
                if r == 0:
                    # k-grouped: all four m-tiles' k0-pair matmuls first (they
                    # need only layer-0 mp0's sigmoid), then the k2-pairs.
                    # Keeps the PE busy while mp1's sigmoid is still running.
                    ps_ab = [px.tile([128, 1024], F32, name="ps")
                             for _ in range(2)]
                    for k in range(0, KH_T, 2):
                        for m in range(4):
                            ps = ps_ab[m // 2]
                            nc.tensor.matmul(
                                ps[:, (m % 2) * NT:(m % 2 + 1) * NT],
                                lhsT=w1c3[:, k:k + 2, m * 128:(m + 1) * 128],
                                rhs=x3[:, k:k + 2, :],
                                start=(k == 0),
                                stop=(k == KH_T - 2),
                                perf_mode=DR,
                            )
                    for mp in range(2):
                        nc.scalar.activation(
                            y_sbs[r][:, mp * 1024:(mp + 1) * 1024],
                            ps_ab[mp][:], TANH, scale=0.5)
                    continue
                for mp in range(2):
                    ps = px.tile([128, 1024], F32, name="ps")
                    for m2 in range(2):
                        m = 2 * mp + m2
                        for k in range(0, KH_T, 2):
                            nc.tensor.matmul(
                                ps[:, m2 * NT:(m2 + 1) * NT],
                                lhsT=w1c3[:, k:k + 2, m * 128:(m + 1) * 128],
                                rhs=x3[:, k:k + 2, :],
                                start=(k == 0),
                                stop=(k == KH_T - 2),
                                perf_mode=DR,
                            )
                    # y2 = tanh(0.5*ps) = 2*sigmoid(ps)-1, stored fp8
                    if r == R_LOC - 1 and mp == 1:
                        # last replica: split the final activation so its
                        # layer 2 can start after the first half
                        nc.scalar.activation(
                            y_sbs[r][:, 1024:1536], ps[:, :512], TANH,
                            scale=0.5)
                        nc.scalar.activation(
                            y_sbs[r][:, 1536:2048], ps[:, 512:], TANH,
                            scale=0.5)
                    else:
                        nc.scalar.activation(
                            y_sbs[r][:, mp * 1024:(mp + 1) * 1024], ps[:],
                            TANH, scale=0.5)
                    if mp == 0:
                        # between this replica's m-pairs: all of the
                        # PREVIOUS replica's layer 2 (both its activations
                        # finished over a full m-pair ago -> no PE stall)
                        l2(r - 1)
                        l2_reduce(r - 1)
                        y_sbs.pop(r - 1)
                        y3s.pop(r - 1)
                        if r == 4:
                            nc.sync.dma_start(
                                out=zt_d[:, :4 * NT], in_=z_sb[:, :4 * NT])
                        if r == 7:
                            nc.sync.dma_start(
                                out=zt_d[:, 4 * NT:7 * NT],
                                in_=z_sb[:, 4 * NT:7 * NT])

            # last replica's layer 2; copy+DMA in two halves so the two
            # receipts pipeline
            r = R_LOC - 1
            l2(r)
            h = NT // 2
            for j in range(2):
                nc.vector.tensor_copy(
                    out=z_sb[:, r * NT + j * h:r * NT + (j + 1) * h],
                    in_=psz[r][0:D_OUT, j * h:(j + 1) * h])
                nc.sync.dma_start(
                    out=zt_d[:, 7 * NT + j * h:7 * NT + (j + 1) * h],
                    in_=z_sb[:, 7 * NT + j * h:7 * NT + (j + 1) * h])

    nc.compile()
    return nc


def kernel(batch, W0, W1, W2, noise1):
    global last_results
    batch = np.asarray(batch, dtype=np.float32)
    W0 = np.asarray(W0, dtype=np.float32)
    W1 = np.asarray(W1, dtype=np.float32)
    W2 = np.asarray(W2, dtype=np.float32)
    noise1 = np.asarray(noise1, dtype=np.float32)

    f8 = mybir.dt.np(FP8)

    A = batch.reshape(BT, D_IN)
    ATp = np.zeros((KA, BT), np.float32)
    ATp[:D_IN] = A.T
    at_full = ATp.reshape(KK0, 2, 128, BT).transpose(2, 0, 1, 3)  # [p,kk,j,n]

    W0p = np.zeros((KA, D_H), np.float32)
    W0p[:D_IN] = W0
    w0_full = W0p.reshape(KK0, 2, 128, D_H).transpose(2, 0, 1, 3)  # [p,kk,j,m]

    noise = noise1.copy()
    noise[0] = 0.0
    W1n = W1[None] + noise                        # [16, 512, 512] fp32

    # w2 pack: [p, (kp j m)] with pack[p, kp*20+j*10+m] = W2[(2kp+j)*128+p, m]
    W2q = W2.astype(f8).astype(np.float32)        # quantized once; b2 matches
    W2qp = np.zeros((D_H, M2), np.float32)
    W2qp[:, :D_OUT] = W2q
    w2_pack = np.ascontiguousarray(
        W2qp.reshape(2, 2, 128, M2).transpose(2, 0, 1, 3).reshape(128, 4 * M2)
    ).astype(f8)
    b2 = 0.5 * W2q.sum(axis=0)                    # [10] host-side bias

    # per-replica-group W1 packs: [p, (r k n)]
    w1_packs = []
    for sg in range(SG):
        blk = W1n[sg * R_LOC:(sg + 1) * R_LOC]    # [8, 512, 512]
        p = blk.reshape(R_LOC, KH_T, 128, D_H).transpose(2, 0, 1, 3)
        w1_packs.append(np.ascontiguousarray(
            p.reshape(128, R_LOC * KH_T * D_H)).astype(f8))

    # per-token-group interleaved A^T|W0 packs: [p, (kk [at|w0])]
    aw_packs = []
    for tg in range(TG):
        at_sl = at_full[:, :, :, tg * NT:(tg + 1) * NT]   # [p, kk, 2, 512]
        aw = np.concatenate(
            [at_sl.reshape(128, KK0, 1024), w0_full.reshape(128, KK0, 1024)],
            axis=2)                                       # [p, kk, 2048]
        aw_packs.append(np.ascontiguousarray(
            aw.reshape(128, KK0 * 2048)).astype(f8))

    in_maps = []
    for c in range(N_CORES):
        sg, tg = c // TG, c % TG
        in_maps.append({
            "aw_pack": aw_packs[tg],
            "w1_pack": w1_packs[sg],
            "w2_pack": w2_pack,
        })

    if "nc" not in _CACHE:
        _CACHE["nc"] = _build_program()
    nc = _CACHE["nc"]

    trace = bool(int(os.environ.get("KERNEL_TRACE", "0")))
    res = run_bass_kernel_spmd(
        nc, in_maps, core_ids=list(range(N_CORES)), trace=trace)
    last_results = res

    out = np.empty((S, BT, D_OUT), np.float32)
    for c in range(N_CORES):
        sg, tg = c // TG, c % TG
        zt = np.asarray(res.results[c]["zt"], dtype=np.float32)  # [10, 8*512]
        for i in range(R_LOC):
            logits = 0.5 * zt[:, i * NT:(i + 1) * NT].T + b2     # [512, 10]
            out[sg * R_LOC + i, tg * NT:(tg + 1) * NT] = (
                1.0 / (1.0 + np.exp(-logits)))
    return out.reshape(S, 32, 64, D_OUT)


# revision 15
# speedup vs baseline: 1.1780x; 1.1780x over previous
"""Trainium2 Bass kernel for the Noisy-Weights BNN MLP.

Computation (full problem):
  noise1[0] = 0;  W1n = W1[None] + noise1            # [16, 512, 512]
  X = sigmoid(A @ W0)        A = batch.reshape(2048, 784)
  Y_s = sigmoid(X @ W1n[s])
  Z_s = sigmoid(Y_s @ W2)    -> out [16, 32, 64, 10]

Sharding over 8 NeuronCores: 2 replica-groups (8 replicas each) x
4 token-groups (512 tokens each).  Each core redundantly computes the
shared layer 0 for its 512 tokens, then its 8 replicas of layers 1+2.

All three layers run in fp8e4m3 with DoubleRow perf mode (2 k-tiles per
pass).  Accuracy is preserved by storing the hidden activation Y in
*centered* form: the layer-1 activation computes y2 = tanh(0.5*ps) =
2*sigmoid(ps)-1, which quantizes to fp8 with half the absolute error of
sigmoid outputs clustered near 1.  Layer 2 then computes y2 @ W2 and the
host finishes with sigmoid(0.5*zt + 0.5*colsum(W2q)).  Simulated
end-to-end rel-L2 error vs the fp32 reference: ~8e-3.

On-device layout: every matmul is out = lhsT.T @ rhs with contraction on
SBUF partitions:
  layer0: lhsT = W0 [128, 2, 128m], rhs = A^T [128, 2, 512] (k-pairs,
          784 zero-padded to 1024 = 4 pairs) -> psum X^T, sigmoid->fp8
  layer1: lhsT = W1n pair,          rhs = X^T pair -> psum, tanh->fp8
  layer2: lhsT = W2 pair [128,2,10], rhs = Y^T pair, both pairs
          accumulate in one PSUM bank -> single DVE copy to bf16

Schedule notes: short dummy matmuls warm the PE clock (HAM) while the
first DMA chunk lands; layer-0 A^T/W0 are packed interleaved per k-pair
and DMA'd in 4 chunks; each replica's layer-2 pair is issued one m-pair
*after* its activation so the PE FIFO never stalls.
"""

import os
import sys

import numpy as np
import ml_dtypes

if "/opt/trn_rl_repo" not in sys.path:
    sys.path.insert(0, "/opt/trn_rl_repo")

import concourse.bass as bass  # noqa: E402
import concourse.tile as tile  # noqa: E402
from concourse import bacc, mybir  # noqa: E402
from concourse.bass_utils import run_bass_kernel_spmd  # noqa: E402

# ---- problem constants (hardcoded; kernel.py must be self-contained) ----
S = 16           # noisy-weight replicas
BT = 2048        # batch tokens = 32 * 64
D_IN = 784
D_H = 512
D_OUT = 10
KA = 896         # 784 zero-padded to 7 * 128 (3 DoubleRow pairs + 1 single)
N_CORES = 8
SG = 2           # replica groups
TG = 4           # token groups
R_LOC = S // SG          # replicas per core = 8
NT = BT // TG            # tokens per core = 512
KP0 = 3                  # layer-0 DoubleRow k-pairs (tiles 0..5)
AW_B = KP0 * 2048 + 1024   # aw pack bytes/partition: 3 pair chunks + single
KH_T = D_H // 128        # 4 k-tiles for hidden dims (2 pairs)

BF16 = mybir.dt.bfloat16
FP8 = mybir.dt.float8e4
F32 = mybir.dt.float32
DR = mybir.MatmulPerfMode.DoubleRow

# layer-2 matmul mode: "dr16" = DoubleRow pairs with W2 zero-padded to 16
# output cols, "plain" = 4 plain fp8 matmuls (ISA forbids dual-fp8
# ldweights at 10 cols)
L2_MODE = os.environ.get("KERNEL_L2_MODE", "dr16")
M2 = 16 if L2_MODE == "dr16" else D_OUT   # layer-2 packed output cols

_CACHE = {}

last_results = None  # BassKernelResults of the most recent run (for test.py)


def _build_program():
    """One SPMD Bass program; per-core differences live entirely in data."""
    nc = bacc.Bacc(None, target_bir_lowering=False, debug=False,
                   enable_partition_id=False)

    # layer-0 inputs interleaved per k-pair:
    # aw[:, kk*2048+0:1024]    = A^T pair [2, 512] (fp8)
    # aw[:, kk*2048+1024:2048] = W0  pair [2, 512] (fp8)
    # trailing single tile 6:  aw[:, 6144:6656] = A^T, aw[:, 6656:7168] = W0
    aw_d = nc.dram_tensor("aw_pack", [128, AW_B], FP8,
                          kind="ExternalInput")
    w1_d = nc.dram_tensor("w1_pack", [128, R_LOC * KH_T * D_H], FP8,
                          kind="ExternalInput")
    w2_d = nc.dram_tensor("w2_pack", [128, KH_T * M2], FP8,
                          kind="ExternalInput")
    zt_d = nc.dram_tensor("zt", [D_OUT, R_LOC * NT], BF16,
                          kind="ExternalOutput")

    SIG = mybir.ActivationFunctionType.Sigmoid
    TANH = mybir.ActivationFunctionType.Tanh

    with tile.TileContext(nc) as tc:
        with (
            tc.tile_pool(name="consts", bufs=1) as consts,
            tc.tile_pool(name="w1p", bufs=1) as w1p,
            tc.tile_pool(name="yp", bufs=3) as yp,
            tc.tile_pool(name="px", bufs=3, space="PSUM") as px,
            tc.tile_pool(name="pz", bufs=2, space="PSUM") as pz,
        ):
            warm_sb = consts.tile([128, 256], FP8)
            aw_sb = consts.tile([128, AW_B], FP8)
            w2_sb = consts.tile([128, KH_T * M2], FP8)
            x_sb = consts.tile([128, KH_T * NT], FP8)
            z_sb = consts.tile([D_OUT, R_LOC * NT], BF16)

            def at_kk(kk):
                return aw_sb[:, kk * 2048:kk * 2048 + 1024].rearrange(
                    "p (a n) -> p a n", a=2)

            def w0_kk(kk):
                return aw_sb[:, kk * 2048 + 1024:(kk + 1) * 2048].rearrange(
                    "p (a n) -> p a n", a=2)

            def w2_kp(kp):
                return w2_sb[:, kp * 2 * M2:(kp + 1) * 2 * M2].rearrange(
                    "p (a m) -> p a m", a=2)

            # All input DMA triggers on the SP queue, in consumption-priority
            # order: the transfers share one effective HBM pipe, so trigger
            # order IS the arrival order.  Issued before the warmups so
            # descriptor generation starts immediately.
            nc.gpsimd.memset(warm_sb[:], 0)   # first: warmups need it
            for kk in range(KP0):
                nc.sync.dma_start(
                    out=aw_sb[:, kk * 2048:(kk + 1) * 2048],
                    in_=aw_d[:, kk * 2048:(kk + 1) * 2048])
            nc.sync.dma_start(out=aw_sb[:, KP0 * 2048:AW_B],
                              in_=aw_d[:, KP0 * 2048:AW_B])
            w1_sb = [w1p.tile([128, 2 * KH_T * D_H], FP8, name=f"w1c{ci}")
                     for ci in range(4)]
            nc.sync.dma_start(
                out=w1_sb[0][:], in_=w1_d[:, 0:2 * KH_T * D_H])
            nc.sync.dma_start(out=w2_sb[:], in_=w2_d[:])
            for ci in range(1, 4):
                nc.sync.dma_start(
                    out=w1_sb[ci][:],
                    in_=w1_d[:, ci * 2 * KH_T * D_H:(ci + 1) * 2 * KH_T * D_H])

            # PE warm-up: short dummy matmuls keep TensorE busy (and
            # un-throttle the HAM clock gate) while the first input DMA
            # lands; short so layer 0 isn't stuck behind them in the FIFO.
            wps = px.tile([128, 1024], F32, name="ps")
            for _ in range(14):
                nc.tensor.matmul(wps[:, :256], lhsT=warm_sb[:, :128],
                                 rhs=warm_sb[:], start=True, stop=True)

            # ---- layer 0: X^T = sigmoid(W0^T A^T) ----
            # mp0 strictly first so its sigmoid (which feeds replica 0's
            # k0-pair matmuls) fires as early as possible; kk-outer so early
            # k-pair chunks are consumed while later chunks are in flight.
            for mp in range(2):           # m pairs: (0,1), (2,3)
                ps = px.tile([128, 1024], F32, name="ps")
                for kk in range(KP0):
                    for m2 in range(2):
                        m = 2 * mp + m2
                        nc.tensor.matmul(
                            ps[:, m2 * NT:(m2 + 1) * NT],
                            lhsT=w0_kk(kk)[:, :, m * 128:(m + 1) * 128],
                            rhs=at_kk(kk),
                            start=(kk == 0),
                            stop=False,
                            perf_mode=DR,
                        )
                for m2 in range(2):       # trailing single k-tile 6 (plain)
                    m = 2 * mp + m2
                    nc.tensor.matmul(
                        ps[:, m2 * NT:(m2 + 1) * NT],
                        lhsT=aw_sb[:, 6656 + m * 128:6656 + (m + 1) * 128],
                        rhs=aw_sb[:, 6144:6656],
                        start=False, stop=True,
                    )
                nc.scalar.activation(
                    x_sb[:, mp * 1024:(mp + 1) * 1024], ps[:], SIG)

            # ---- per replica: layer 1, with layer 2 deferred-interleaved ----
            psz = {}

            def l2(r):
                # all k-tiles accumulate into one PSUM bank
                if L2_MODE == "dr16":
                    for kp in range(2):
                        nc.tensor.matmul(
                            psz[r][0:M2, :],
                            lhsT=w2_kp(kp),
                            rhs=y3s[r][:, 2 * kp:2 * kp + 2, :],
                            start=(kp == 0), stop=(kp == 1),
                            perf_mode=DR,
                        )
                else:
                    for k in range(KH_T):
                        nc.tensor.matmul(
                            psz[r][0:D_OUT, :],
                            lhsT=w2_sb[:, k * M2:k * M2 + D_OUT],
                            rhs=y_sbs[r][:, k * NT:(k + 1) * NT],
                            start=(k == 0), stop=(k == KH_T - 1),
                        )

            def l2_reduce(r):
                nc.vector.tensor_copy(out=z_sb[:, r * NT:(r + 1) * NT],
                                      in_=psz[r][0:D_OUT, :])
                psz.pop(r)

            y_sbs = {}
            y3s = {}
            x3 = x_sb[:].rearrange("p (k n) -> p k n", k=KH_T)
            for r in range(R_LOC):
                w1c = w1_sb[r // 2]
                roff = (r % 2) * KH_T * D_H
                w1c3 = w1c[:, roff:roff + KH_T * D_H].rearrange(
                    "p (k n) -> p k n", k=KH_T)
                y_sbs[r] = yp.tile([128, KH_T * NT], FP8, name="y_sb")
                y3s[r] = y_sbs[r][:].rearrange("p (k n) -> p k n", k=KH_T)
                psz[r] = pz.tile([128, NT], F32, name="psz")
                if r == 0:
                    # k-grouped: all four m-tiles' k0-pair matmuls first (they
                    # need only layer-0 mp0's sigmoid), then the k2-pairs.
                    # Keeps the PE busy while mp1's sigmoid is still running.
                    ps_ab = [px.tile([128, 1024], F32, name="ps")
                             for _ in range(2)]
                    for k in range(0, KH_T, 2):
                        for m in range(4):
                            ps = ps_ab[m // 2]
                            nc.tensor.matmul(
                                ps[:, (m % 2) * NT:(m % 2 + 1) * NT],
                                lhsT=w1c3[:, k:k + 2, m * 128:(m + 1) * 128],
                                rhs=x3[:, k:k + 2, :],
                                start=(k == 0),
                                stop=(k == KH_T - 2),
                                perf_mode=DR,
                            )
                    for mp in range(2):
                        nc.scalar.activation(
                            y_sbs[r][:, mp * 1024:(mp + 1) * 1024],
                            ps_ab[mp][:], TANH, scale=0.5)
                    continue
                for mp in range(2):
                    ps = px.tile([128, 1024], F32, name="ps")
                    for m2 in range(2):
                        m = 2 * mp + m2
                        for k in range(0, KH_T, 2):
                            nc.tensor.matmul(
                                ps[:, m2 * NT:(m2 + 1) * NT],
                                lhsT=w1c3[:, k:k + 2, m * 128:(m + 1) * 128],
                                rhs=x3[:, k:k + 2, :],
                                start=(k == 0),
                                stop=(k == KH_T - 2),
                                perf_mode=DR,
                            )
                    # y2 = tanh(0.5*ps) = 2*sigmoid(ps)-1, stored fp8
                    if r == R_LOC - 1 and mp == 1:
                        # last replica: split the final activation so its
                        # layer 2 can start after the first half
                        nc.scalar.activation(
                            y_sbs[r][:, 1024:1536], ps[:, :512], TANH,
                            scale=0.5)
                        nc.scalar.activation(
                            y_sbs[r][:, 1536:2048], ps[:, 512:], TANH,
                            scale=0.5)
                    else:
                        nc.scalar.activation(
                            y_sbs[r][:, mp * 1024:(mp + 1) * 1024], ps[:],
                            TANH, scale=0.5)
                    if mp == 0:
                        # between this replica's m-pairs: all of the
                        # PREVIOUS replica's layer 2 (both its activations
                        # finished over a full m-pair ago -> no PE stall)
                        l2(r - 1)
                        l2_reduce(r - 1)
                        y_sbs.pop(r - 1)
                        y3s.pop(r - 1)
                        if r == 4:
                            nc.sync.dma_start(
                                out=zt_d[:, :4 * NT], in_=z_sb[:, :4 * NT])
                        if r == 7:
                            nc.sync.dma_start(
                                out=zt_d[:, 4 * NT:7 * NT],
                                in_=z_sb[:, 4 * NT:7 * NT])

            # last replica's layer 2; copy+DMA in two halves so the two
            # receipts pipeline
            r = R_LOC - 1
            l2(r)
            h = NT // 2
            for j in range(2):
                nc.vector.tensor_copy(
                    out=z_sb[:, r * NT + j * h:r * NT + (j + 1) * h],
                    in_=psz[r][0:D_OUT, j * h:(j + 1) * h])
                nc.sync.dma_start(
                    out=zt_d[:, 7 * NT + j * h:7 * NT + (j + 1) * h],
                    in_=z_sb[:, 7 * NT + j * h:7 * NT + (j + 1) * h])

    nc.compile()
    return nc


def kernel(batch, W0, W1, W2, noise1):
    global last_results
    batch = np.asarray(batch, dtype=np.float32)
    W0 = np.asarray(W0, dtype=np.float32)
    W1 = np.asarray(W1, dtype=np.float32)
    W2 = np.asarray(W2, dtype=np.float32)
    noise1 = np.asarray(noise1, dtype=np.float32)

    f8 = mybir.dt.np(FP8)

    A = batch.reshape(BT, D_IN)
    ATp = np.zeros((KA, BT), np.float32)
    ATp[:D_IN] = A.T
    W0p = np.zeros((KA, D_H), np.float32)
    W0p[:D_IN] = W0
    # pairs: tiles 0..5 -> [p, kk, j, n]; single: tile 6 -> [p, n]
    at_pair = ATp[:768].reshape(KP0, 2, 128, BT).transpose(2, 0, 1, 3)
    w0_pair = W0p[:768].reshape(KP0, 2, 128, D_H).transpose(2, 0, 1, 3)
    at_sing = ATp[768:].reshape(128, BT)
    w0_sing = W0p[768:].reshape(128, D_H)

    noise = noise1.copy()
    noise[0] = 0.0
    W1n = W1[None] + noise                        # [16, 512, 512] fp32

    # w2 pack: [p, (kp j m)] with pack[p, kp*20+j*10+m] = W2[(2kp+j)*128+p, m]
    W2q = W2.astype(f8).astype(np.float32)        # quantized once; b2 matches
    W2qp = np.zeros((D_H, M2), np.float32)
    W2qp[:, :D_OUT] = W2q
    w2_pack = np.ascontiguousarray(
        W2qp.reshape(2, 2, 128, M2).transpose(2, 0, 1, 3).reshape(128, 4 * M2)
    ).astype(f8)
    b2 = 0.5 * W2q.sum(axis=0)                    # [10] host-side bias

    # per-replica-group W1 packs: [p, (r k n)]
    w1_packs = []
    for sg in range(SG):
        blk = W1n[sg * R_LOC:(sg + 1) * R_LOC]    # [8, 512, 512]
        p = blk.reshape(R_LOC, KH_T, 128, D_H).transpose(2, 0, 1, 3)
        w1_packs.append(np.ascontiguousarray(
            p.reshape(128, R_LOC * KH_T * D_H)).astype(f8))

    # per-token-group interleaved A^T|W0 packs:
    # [p, (kk [at|w0]) ... at_single w0_single]
    aw_packs = []
    for tg in range(TG):
        tsl = slice(tg * NT, (tg + 1) * NT)
        at_sl = at_pair[:, :, :, tsl]                     # [p, kk, 2, 512]
        aw = np.concatenate(
            [at_sl.reshape(128, KP0, 1024), w0_pair.reshape(128, KP0, 1024)],
            axis=2).reshape(128, KP0 * 2048)              # [p, kk*2048]
        aw = np.concatenate([aw, at_sing[:, tsl], w0_sing], axis=1)
        aw_packs.append(np.ascontiguousarray(aw).astype(f8))

    in_maps = []
    for c in range(N_CORES):
        sg, tg = c // TG, c % TG
        in_maps.append({
            "aw_pack": aw_packs[tg],
            "w1_pack": w1_packs[sg],
            "w2_pack": w2_pack,
        })

    if "nc" not in _CACHE:
        _CACHE["nc"] = _build_program()
    nc = _CACHE["nc"]

    trace = bool(int(os.environ.get("KERNEL_TRACE", "0")))
    res = run_bass_kernel_spmd(
        nc, in_maps, core_ids=list(range(N_CORES)), trace=trace)
    last_results = res

    out = np.empty((S, BT, D_OUT), np.float32)
    for c in range(N_CORES):
        sg, tg = c // TG, c % TG
        zt = np.asarray(res.results[c]["zt"], dtype=np.float32)  # [10, 8*512]
        for i in range(R_LOC):
            logits = 0.5 * zt[:, i * NT:(i + 1) * NT].T + b2     # [512, 10]
            out[sg * R_LOC + i, tg * NT:(tg + 1) * NT] = (
                1.0 / (1.0 + np.exp(-logits)))
    return out.reshape(S, 32, 64, D_OUT)


# revision 17
# speedup vs baseline: 1.2143x; 1.0308x over previous
"""Trainium2 Bass kernel for the Noisy-Weights BNN MLP.

Computation (full problem):
  noise1[0] = 0;  W1n = W1[None] + noise1            # [16, 512, 512]
  X = sigmoid(A @ W0)        A = batch.reshape(2048, 784)
  Y_s = sigmoid(X @ W1n[s])
  Z_s = sigmoid(Y_s @ W2)    -> out [16, 32, 64, 10]

Sharding over 8 NeuronCores: 2 replica-groups (8 replicas each) x
4 token-groups (512 tokens each).  Each core redundantly computes the
shared layer 0 for its 512 tokens, then its 8 replicas of layers 1+2.

All three layers run in fp8e4m3 with DoubleRow perf mode (2 k-tiles per
pass).  Accuracy is preserved by storing the hidden activation Y in
*centered* form: the layer-1 activation computes y2 = tanh(0.5*ps) =
2*sigmoid(ps)-1, which quantizes to fp8 with half the absolute error of
sigmoid outputs clustered near 1.  Layer 2 then computes y2 @ W2 and the
host finishes with sigmoid(0.5*zt + 0.5*colsum(W2q)).  Simulated
end-to-end rel-L2 error vs the fp32 reference: ~8e-3.

On-device layout: every matmul is out = lhsT.T @ rhs with contraction on
SBUF partitions:
  layer0: lhsT = W0 [128, 2, 128m], rhs = A^T [128, 2, 512] (k-pairs,
          784 zero-padded to 1024 = 4 pairs) -> psum X^T, sigmoid->fp8
  layer1: lhsT = W1n pair,          rhs = X^T pair -> psum, tanh->fp8
  layer2: lhsT = W2 pair [128,2,10], rhs = Y^T pair, both pairs
          accumulate in one PSUM bank -> single DVE copy to bf16

Schedule notes: short dummy matmuls warm the PE clock (HAM) while the
first DMA chunk lands; layer-0 A^T/W0 are packed interleaved per k-pair
and DMA'd in 4 chunks; each replica's layer-2 pair is issued one m-pair
*after* its activation so the PE FIFO never stalls.
"""

import os
import sys

import numpy as np
import ml_dtypes

if "/opt/trn_rl_repo" not in sys.path:
    sys.path.insert(0, "/opt/trn_rl_repo")

import concourse.bass as bass  # noqa: E402
import concourse.tile as tile  # noqa: E402
from concourse import bacc, mybir  # noqa: E402
from concourse.bass_utils import run_bass_kernel_spmd  # noqa: E402

# ---- problem constants (hardcoded; kernel.py must be self-contained) ----
S = 16           # noisy-weight replicas
BT = 2048        # batch tokens = 32 * 64
D_IN = 784
D_H = 512
D_OUT = 10
KA = 896         # 784 zero-padded to 7 * 128 (3 DoubleRow pairs + 1 single)
N_CORES = 8
SG = 2           # replica groups
TG = 4           # token groups
R_LOC = S // SG          # replicas per core = 8
NT = BT // TG            # tokens per core = 512
KP0 = 3                  # layer-0 DoubleRow k-pairs (tiles 0..5)
AW_B = KP0 * 2048 + 1024   # aw pack bytes/partition: 3 pair chunks + single
KH_T = D_H // 128        # 4 k-tiles for hidden dims (2 pairs)

BF16 = mybir.dt.bfloat16
FP8 = mybir.dt.float8e4
F32 = mybir.dt.float32
DR = mybir.MatmulPerfMode.DoubleRow

# layer-2 matmul mode: "dr16" = DoubleRow pairs with W2 zero-padded to 16
# output cols, "plain" = 4 plain fp8 matmuls (ISA forbids dual-fp8
# ldweights at 10 cols)
L2_MODE = os.environ.get("KERNEL_L2_MODE", "dr16")
M2 = 16 if L2_MODE == "dr16" else D_OUT   # layer-2 packed output cols

_CACHE = {}

last_results = None  # BassKernelResults of the most recent run (for test.py)


def _build_program():
    """One SPMD Bass program; per-core differences live entirely in data."""
    nc = bacc.Bacc(None, target_bir_lowering=False, debug=False,
                   enable_partition_id=False)

    # layer-0 inputs interleaved per k-pair:
    # aw[:, kk*2048+0:1024]    = A^T pair [2, 512] (fp8)
    # aw[:, kk*2048+1024:2048] = W0  pair [2, 512] (fp8)
    # trailing single tile 6:  aw[:, 6144:6656] = A^T, aw[:, 6656:7168] = W0
    aw_d = nc.dram_tensor("aw_pack", [128, AW_B], FP8,
                          kind="ExternalInput")
    w1_d = nc.dram_tensor("w1_pack", [128, R_LOC * KH_T * D_H], FP8,
                          kind="ExternalInput")
    w2_d = nc.dram_tensor("w2_pack", [128, KH_T * M2], FP8,
                          kind="ExternalInput")
    zt_d = nc.dram_tensor("zt", [D_OUT, R_LOC * NT], BF16,
                          kind="ExternalOutput")

    SIG = mybir.ActivationFunctionType.Sigmoid
    TANH = mybir.ActivationFunctionType.Tanh

    with tile.TileContext(nc) as tc:
        with (
            tc.tile_pool(name="consts", bufs=1) as consts,
            tc.tile_pool(name="w1p", bufs=1) as w1p,
            tc.tile_pool(name="yp", bufs=3) as yp,
            tc.tile_pool(name="px", bufs=3, space="PSUM") as px,
            tc.tile_pool(name="pz", bufs=2, space="PSUM") as pz,
        ):
            warm_sb = consts.tile([128, 256], FP8)
            aw_sb = consts.tile([128, AW_B], FP8)
            w2_sb = consts.tile([128, KH_T * M2], FP8)
            x_sb = consts.tile([128, KH_T * NT], FP8)
            z_sb = consts.tile([D_OUT, R_LOC * NT], BF16)

            def at_kk(kk):
                return aw_sb[:, kk * 2048:kk * 2048 + 1024].rearrange(
                    "p (a n) -> p a n", a=2)

            def w0_kk(kk):
                return aw_sb[:, kk * 2048 + 1024:(kk + 1) * 2048].rearrange(
                    "p (a n) -> p a n", a=2)

            def w2_kp(kp):
                return w2_sb[:, kp * 2 * M2:(kp + 1) * 2 * M2].rearrange(
                    "p (a m) -> p a m", a=2)

            # Input DMA triggers in consumption-priority order, alternating
            # between the two HWDGE queues (SP, Activation) so the two
            # hardware DMA paths can overlap transfers.  Per-replica W1
            # chunks so replica r only waits for its own 0.25 MiB.
            nc.gpsimd.memset(warm_sb[:], 0)   # first: warmups need it
            w1_sb = [w1p.tile([128, KH_T * D_H], FP8, name=f"w1r{r}")
                     for r in range(R_LOC)]
            in_dmas = []
            for kk in range(KP0):
                in_dmas.append((aw_sb[:, kk * 2048:(kk + 1) * 2048],
                                aw_d[:, kk * 2048:(kk + 1) * 2048]))
            in_dmas.append((aw_sb[:, KP0 * 2048:AW_B],
                            aw_d[:, KP0 * 2048:AW_B]))
            in_dmas.append((w1_sb[0][:], w1_d[:, 0:KH_T * D_H]))
            in_dmas.append((w2_sb[:], w2_d[:]))
            for r in range(1, R_LOC):
                in_dmas.append((
                    w1_sb[r][:],
                    w1_d[:, r * KH_T * D_H:(r + 1) * KH_T * D_H]))
            for j, (out_ap, in_ap) in enumerate(in_dmas):
                eng = nc.sync if j % 2 == 0 else nc.scalar
                eng.dma_start(out=out_ap, in_=in_ap)

            # PE warm-up: short dummy matmuls keep TensorE busy (and
            # un-throttle the HAM clock gate) while the first input DMA
            # lands; short so layer 0 isn't stuck behind them in the FIFO.
            wps = px.tile([128, 1024], F32, name="ps")
            for _ in range(14):
                nc.tensor.matmul(wps[:, :256], lhsT=warm_sb[:, :128],
                                 rhs=warm_sb[:], start=True, stop=True)

            # ---- layer 0: X^T = sigmoid(W0^T A^T) ----
            # mp0 strictly first so its sigmoid (which feeds replica 0's
            # k0-pair matmuls) fires as early as possible; kk-outer so early
            # k-pair chunks are consumed while later chunks are in flight.
            for mp in range(2):           # m pairs: (0,1), (2,3)
                ps = px.tile([128, 1024], F32, name="ps")
                for kk in range(KP0):
                    for m2 in range(2):
                        m = 2 * mp + m2
                        nc.tensor.matmul(
                            ps[:, m2 * NT:(m2 + 1) * NT],
                            lhsT=w0_kk(kk)[:, :, m * 128:(m + 1) * 128],
                            rhs=at_kk(kk),
                            start=(kk == 0),
                            stop=False,
                            perf_mode=DR,
                        )
                for m2 in range(2):       # trailing single k-tile 6 (plain)
                    m = 2 * mp + m2
                    nc.tensor.matmul(
                        ps[:, m2 * NT:(m2 + 1) * NT],
                        lhsT=aw_sb[:, 6656 + m * 128:6656 + (m + 1) * 128],
                        rhs=aw_sb[:, 6144:6656],
                        start=False, stop=True,
                    )
                nc.scalar.activation(
                    x_sb[:, mp * 1024:(mp + 1) * 1024], ps[:], SIG)

            # ---- per replica: layer 1, with layer 2 deferred-interleaved ----
            psz = {}

            def l2(r):
                # all k-tiles accumulate into one PSUM bank
                if L2_MODE == "dr16":
                    for kp in range(2):
                        nc.tensor.matmul(
                            psz[r][0:M2, :],
                            lhsT=w2_kp(kp),
                            rhs=y3s[r][:, 2 * kp:2 * kp + 2, :],
                            start=(kp == 0), stop=(kp == 1),
                            perf_mode=DR,
                        )
                else:
                    for k in range(KH_T):
                        nc.tensor.matmul(
                            psz[r][0:D_OUT, :],
                            lhsT=w2_sb[:, k * M2:k * M2 + D_OUT],
                            rhs=y_sbs[r][:, k * NT:(k + 1) * NT],
                            start=(k == 0), stop=(k == KH_T - 1),
                        )

            def l2_reduce(r):
                nc.vector.tensor_copy(out=z_sb[:, r * NT:(r + 1) * NT],
                                      in_=psz[r][0:D_OUT, :])
                psz.pop(r)

            y_sbs = {}
            y3s = {}
            x3 = x_sb[:].rearrange("p (k n) -> p k n", k=KH_T)
            for r in range(R_LOC):
                w1c3 = w1_sb[r][:].rearrange("p (k n) -> p k n", k=KH_T)
                y_sbs[r] = yp.tile([128, KH_T * NT], FP8, name="y_sb")
                y3s[r] = y_sbs[r][:].rearrange("p (k n) -> p k n", k=KH_T)
                psz[r] = pz.tile([128, NT], F32, name="psz")
                if r == 0:
                    # k-grouped: all four m-tiles' k0-pair matmuls first (they
                    # need only layer-0 mp0's sigmoid), then the k2-pairs.
                    # Keeps the PE busy while mp1's sigmoid is still running.
                    ps_ab = [px.tile([128, 1024], F32, name="ps")
                             for _ in range(2)]
                    for k in range(0, KH_T, 2):
                        for m in range(4):
                            ps = ps_ab[m // 2]
                            nc.tensor.matmul(
                                ps[:, (m % 2) * NT:(m % 2 + 1) * NT],
                                lhsT=w1c3[:, k:k + 2, m * 128:(m + 1) * 128],
                                rhs=x3[:, k:k + 2, :],
                                start=(k == 0),
                                stop=(k == KH_T - 2),
                                perf_mode=DR,
                            )
                    for mp in range(2):
                        nc.scalar.activation(
                            y_sbs[r][:, mp * 1024:(mp + 1) * 1024],
                            ps_ab[mp][:], TANH, scale=0.5)
                    continue
                for mp in range(2):
                    ps = px.tile([128, 1024], F32, name="ps")
                    for m2 in range(2):
                        m = 2 * mp + m2
                        for k in range(0, KH_T, 2):
                            nc.tensor.matmul(
                                ps[:, m2 * NT:(m2 + 1) * NT],
                                lhsT=w1c3[:, k:k + 2, m * 128:(m + 1) * 128],
                                rhs=x3[:, k:k + 2, :],
                                start=(k == 0),
                                stop=(k == KH_T - 2),
                                perf_mode=DR,
                            )
                    # y2 = tanh(0.5*ps) = 2*sigmoid(ps)-1, stored fp8
                    if r == R_LOC - 1 and mp == 1:
                        # last replica: split the final activation so its
                        # layer 2 can start after the first half
                        nc.scalar.activation(
                            y_sbs[r][:, 1024:1536], ps[:, :512], TANH,
                            scale=0.5)
                        nc.scalar.activation(
                            y_sbs[r][:, 1536:2048], ps[:, 512:], TANH,
                            scale=0.5)
                    else:
                        nc.scalar.activation(
                            y_sbs[r][:, mp * 1024:(mp + 1) * 1024], ps[:],
                            TANH, scale=0.5)
                    if mp == 0:
                        # between this replica's m-pairs: all of the
                        # PREVIOUS replica's layer 2 (both its activations
                        # finished over a full m-pair ago -> no PE stall)
                        l2(r - 1)
                        l2_reduce(r - 1)
                        y_sbs.pop(r - 1)
                        y3s.pop(r - 1)
                        if r == 4:
                            nc.sync.dma_start(
                                out=zt_d[:, :4 * NT], in_=z_sb[:, :4 * NT])
                        if r == 7:
                            nc.sync.dma_start(
                                out=zt_d[:, 4 * NT:7 * NT],
                                in_=z_sb[:, 4 * NT:7 * NT])

            # last replica's layer 2; copy+DMA in two halves so the two
            # receipts pipeline
            r = R_LOC - 1
            l2(r)
            h = NT // 2
            for j in range(2):
                nc.vector.tensor_copy(
                    out=z_sb[:, r * NT + j * h:r * NT + (j + 1) * h],
                    in_=psz[r][0:D_OUT, j * h:(j + 1) * h])
                nc.sync.dma_start(
                    out=zt_d[:, 7 * NT + j * h:7 * NT + (j + 1) * h],
                    in_=z_sb[:, 7 * NT + j * h:7 * NT + (j + 1) * h])

    nc.compile()
    return nc


def kernel(batch, W0, W1, W2, noise1):
    global last_results
    batch = np.asarray(batch, dtype=np.float32)
    W0 = np.asarray(W0, dtype=np.float32)
    W1 = np.asarray(W1, dtype=np.float32)
    W2 = np.asarray(W2, dtype=np.float32)
    noise1 = np.asarray(noise1, dtype=np.float32)

    f8 = mybir.dt.np(FP8)

    A = batch.reshape(BT, D_IN)
    ATp = np.zeros((KA, BT), np.float32)
    ATp[:D_IN] = A.T
    W0p = np.zeros((KA, D_H), np.float32)
    W0p[:D_IN] = W0
    # pairs: tiles 0..5 -> [p, kk, j, n]; single: tile 6 -> [p, n]
    at_pair = ATp[:768].reshape(KP0, 2, 128, BT).transpose(2, 0, 1, 3)
    w0_pair = W0p[:768].reshape(KP0, 2, 128, D_H).transpose(2, 0, 1, 3)
    at_sing = ATp[768:].reshape(128, BT)
    w0_sing = W0p[768:].reshape(128, D_H)

    noise = noise1.copy()
    noise[0] = 0.0
    W1n = W1[None] + noise                        # [16, 512, 512] fp32

    # w2 pack: [p, (kp j m)] with pack[p, kp*20+j*10+m] = W2[(2kp+j)*128+p, m]
    W2q = W2.astype(f8).astype(np.float32)        # quantized once; b2 matches
    W2qp = np.zeros((D_H, M2), np.float32)
    W2qp[:, :D_OUT] = W2q
    w2_pack = np.ascontiguousarray(
        W2qp.reshape(2, 2, 128, M2).transpose(2, 0, 1, 3).reshape(128, 4 * M2)
    ).astype(f8)
    b2 = 0.5 * W2q.sum(axis=0)                    # [10] host-side bias

    # per-replica-group W1 packs: [p, (r k n)]
    w1_packs = []
    for sg in range(SG):
        blk = W1n[sg * R_LOC:(sg + 1) * R_LOC]    # [8, 512, 512]
        p = blk.reshape(R_LOC, KH_T, 128, D_H).transpose(2, 0, 1, 3)
        w1_packs.append(np.ascontiguousarray(
            p.reshape(128, R_LOC * KH_T * D_H)).astype(f8))

    # per-token-group interleaved A^T|W0 packs:
    # [p, (kk [at|w0]) ... at_single w0_single]
    aw_packs = []
    for tg in range(TG):
        tsl = slice(tg * NT, (tg + 1) * NT)
        at_sl = at_pair[:, :, :, tsl]                     # [p, kk, 2, 512]
        aw = np.concatenate(
            [at_sl.reshape(128, KP0, 1024), w0_pair.reshape(128, KP0, 1024)],
            axis=2).reshape(128, KP0 * 2048)              # [p, kk*2048]
        aw = np.concatenate([aw, at_sing[:, tsl], w0_sing], axis=1)
        aw_packs.append(np.ascontiguousarray(aw).astype(f8))

    in_maps = []
    for c in range(N_CORES):
        sg, tg = c // TG, c % TG
        in_maps.append({
            "aw_pack": aw_packs[tg],
            "w1_pack": w1_packs[sg],
            "w2_pack": w2_pack,
        })

    if "nc" not in _CACHE:
        _CACHE["nc"] = _build_program()
    nc = _CACHE["nc"]

    trace = bool(int(os.environ.get("KERNEL_TRACE", "0")))
    res = run_bass_kernel_spmd(
        nc, in_maps, core_ids=list(range(N_CORES)), trace=trace)
    last_results = res

    out = np.empty((S, BT, D_OUT), np.float32)
    for c in range(N_CORES):
        sg, tg = c // TG, c % TG
        zt = np.asarray(res.results[c]["zt"], dtype=np.float32)  # [10, 8*512]
        for i in range(R_LOC):
            logits = 0.5 * zt[:, i * NT:(i + 1) * NT].T + b2     # [512, 10]
            out[sg * R_LOC + i, tg * NT:(tg + 1) * NT] = (
                1.0 / (1.0 + np.exp(-logits)))
    return out.reshape(S, 32, 64, D_OUT)
